# Initial kernel scaffold
#
"""AttentionBlock kernel for 8 TRN2 NeuronCores.

Reference math (per sample s of 4, C=256 channels, HW=64*64=4096 positions):
  qkv = w_qkv @ x + b_qkv ; q,k,v = split(qkv)
  S = (q^T k) / sqrt(C) ; P = softmax(S, axis=-1)
  out = w_out @ (P @ v^T)^T + b_out + x

Sharding: core i -> (sample s=i//2, row half h=i%2, rows n0=h*2048 .. +2048).
K/V are computed for the full sample on both half-cores (duplicate compute is
cheap); Q and the attention rows only for the core's half.

On-chip layout: scores are computed transposed, S^T[m, n] (m = key position on
partitions, n = query row in free dim), so P^T = exp(S^T) is directly the
moving operand of the PV matmul (contraction over m = partitions) -- no
transposes anywhere. Softmax row sums come from an extra matmul with an
all-ones stationary operand (result is pre-broadcast across partitions);
normalization is folded into the PSUM->SBUF copy as a tensor*tensor multiply
with the reciprocal. Projection biases are folded into the matmuls by
augmenting x and the weights with a ones row (K=257); the 1/sqrt(C) scale is
folded into w_q on the host.
"""

import sys

sys.path.insert(0, "/opt/trn_rl_repo")

import numpy as np

import concourse.bacc as bacc
import concourse.tile as tile
from concourse import mybir
from concourse.bass_utils import run_bass_kernel_spmd

B = 4
C = 256
HW = 4096  # 64*64
NH = 2048  # rows per core (half sample)
NT = 512   # n-tile (free dim per psum bank)
F32 = mybir.dt.float32
F32R = mybir.dt.float32r

_CACHE = {}


def _r(ap):
    return ap.bitcast(F32R)


def _emit(nc):
    xkv = nc.dram_tensor("xkv", (C + 1, HW), F32, kind="ExternalInput")
    xq = nc.dram_tensor("xq", (C + 1, NH), F32, kind="ExternalInput")
    wq = nc.dram_tensor("wq", (C + 1, C), F32, kind="ExternalInput")
    wk = nc.dram_tensor("wk", (C + 1, C), F32, kind="ExternalInput")
    wv = nc.dram_tensor("wv", (C + 1, C), F32, kind="ExternalInput")
    wo = nc.dram_tensor("wo", (C + 1, C), F32, kind="ExternalInput")
    y = nc.dram_tensor("y", (C, NH), F32, kind="ExternalOutput")

    with tile.TileContext(nc) as tc:
        with (
            tc.tile_pool(name="const", bufs=1) as const,
            tc.tile_pool(name="xp", bufs=1) as xp,
            tc.tile_pool(name="qk", bufs=1) as qk,
            tc.tile_pool(name="vt", bufs=1) as vtp,
            tc.tile_pool(name="pt", bufs=4) as ptp,
            tc.tile_pool(name="ep", bufs=2) as ep,
        ):
            # ---- load inputs ----
            xk0 = xp.tile([128, HW], F32, tag="xk0")
            xk1 = xp.tile([128, HW], F32, tag="xk1")
            xon = xp.tile([1, HW], F32, tag="xon")
            nc.sync.dma_start(out=xk0, in_=xkv.ap()[0:128, :])
            nc.sync.dma_start(out=xk1, in_=xkv.ap()[128:256, :])
            nc.sync.dma_start(out=xon, in_=xkv.ap()[256:257, :])

            xq0 = xp.tile([128, NH], F32, tag="xq0")
            xq1 = xp.tile([128, NH], F32, tag="xq1")
            xqon = xp.tile([1, NH], F32, tag="xqon")
            nc.sync.dma_start(out=xq0, in_=xq.ap()[0:128, :])
            nc.sync.dma_start(out=xq1, in_=xq.ap()[128:256, :])
            nc.sync.dma_start(out=xqon, in_=xq.ap()[256:257, :])

            ws = {}
            for name, t in (("wq", wq), ("wk", wk), ("wv", wv), ("wo", wo)):
                w0 = const.tile([128, C], F32, tag=name + "0")
                w1 = const.tile([128, C], F32, tag=name + "1")
                wb = const.tile([1, C], F32, tag=name + "b")
                nc.sync.dma_start(out=w0, in_=t.ap()[0:128, :])
                nc.sync.dma_start(out=w1, in_=t.ap()[128:256, :])
                nc.sync.dma_start(out=wb, in_=t.ap()[256:257, :])
                ws[name] = (w0, w1, wb)

            ones128 = const.tile([128, 128], F32, tag="ones")
            nc.vector.memset(ones128, 1.0)

            # ---- projections ----
            # q[cb][128, NH] (pre-scaled by 1/sqrt(C) via host-scaled wq)
            q_sb = [qk.tile([128, NH], F32, tag=f"q{cb}") for cb in range(2)]
            k_sb = [qk.tile([128, HW], F32, tag=f"k{cb}") for cb in range(2)]
            # vT[mb][128, C]: vT[m, c] = sum_ci x_aug[ci, m] * wv_aug[ci, c]
            vt_sb = [vtp.tile([128, C], F32, tag=f"v{mb}") for mb in range(HW // 128)]

            with tc.tile_pool(name="pj", bufs=2, space="PSUM") as pj:
                w0, w1, wb = ws["wq"]
                for cb in range(2):
                    for half in range(2):
                        sl = slice(half * 1024, half * 1024 + 1024)
                        ps = pj.tile([128, 1024], F32, tag="pp")
                        for j in range(2):
                            s2 = slice(half * 1024 + j * 512, half * 1024 + j * 512 + 512)
                            o2 = slice(j * 512, j * 512 + 512)
                            cs = slice(cb * 128, cb * 128 + 128)
                            nc.tensor.matmul(ps[:, o2], _r(w0[:, cs]), _r(xq0[:, s2]),
                                             start=True, stop=False)
                            nc.tensor.matmul(ps[:, o2], _r(w1[:, cs]), _r(xq1[:, s2]),
                                             start=False, stop=False)
                            nc.tensor.matmul(ps[:, o2], _r(wb[0:1, cs]), _r(xqon[0:1, s2]),
                                             start=False, stop=True)
                        nc.vector.tensor_copy(q_sb[cb][:, sl], ps)

                w0, w1, wb = ws["wk"]
                xks = (xk0, xk1, xon)
                for cb in range(2):
                    for quad in range(4):
                        sl = slice(quad * 1024, quad * 1024 + 1024)
                        ps = pj.tile([128, 1024], F32, tag="pp")
                        for j in range(2):
                            s2 = slice(quad * 1024 + j * 512, quad * 1024 + j * 512 + 512)
                            o2 = slice(j * 512, j * 512 + 512)
                            cs = slice(cb * 128, cb * 128 + 128)
                            nc.tensor.matmul(ps[:, o2], _r(w0[:, cs]), _r(xk0[:, s2]),
                                             start=True, stop=False)
                            nc.tensor.matmul(ps[:, o2], _r(w1[:, cs]), _r(xk1[:, s2]),
                                             start=False, stop=False)
                            nc.tensor.matmul(ps[:, o2], _r(wb[0:1, cs]), _r(xon[0:1, s2]),
                                             start=False, stop=True)
                        nc.scalar.copy(k_sb[cb][:, sl], ps)

                w0, w1, wb = ws["wv"]
                for mb in range(HW // 128):
                    ms = slice(mb * 128, mb * 128 + 128)
                    pv = pj.tile([128, C], F32, tag="pv")
                    nc.tensor.matmul(pv, _r(xk0[:, ms]), _r(w0), start=True, stop=False)
                    nc.tensor.matmul(pv, _r(xk1[:, ms]), _r(w1), start=False, stop=False)
                    nc.tensor.matmul(pv, _r(xon[0:1, ms]), _r(wb[0:1, :]),
                                     start=False, stop=True)
                    if mb % 2 == 0:
                        nc.vector.tensor_copy(vt_sb[mb], pv)
                    else:
                        nc.scalar.copy(vt_sb[mb], pv)

            # ---- attention main loop ----
            wo0, wo1, wob = ws["wo"]
            n_mp = HW // 256  # m-pairs of 128 rows each
            with (
                tc.tile_pool(name="pss", bufs=2, space="PSUM") as pss,
                tc.tile_pool(name="pacc", bufs=1, space="PSUM") as pacc,
                tc.tile_pool(name="pf", bufs=1, space="PSUM") as pf,
            ):
                for nt in range(NH // NT):
                    nsl = slice(nt * NT, nt * NT + NT)
                    po = [pacc.tile([128, NT], F32, tag=f"po{cb}") for cb in range(2)]
                    psum = pacc.tile([128, NT], F32, tag="psum")
                    for mp in range(n_mp):
                        ps = pss.tile([128, 2 * NT], F32, tag="ps")
                        for j in range(2):  # two m-chunks per pair
                            mb = 2 * mp + j
                            msl = slice(mb * 128, mb * 128 + 128)
                            osl = slice(j * NT, j * NT + NT)
                            nc.tensor.matmul(ps[:, osl], _r(k_sb[0][:, msl]),
                                             _r(q_sb[0][:, nsl]), start=True, stop=False)
                            nc.tensor.matmul(ps[:, osl], _r(k_sb[1][:, msl]),
                                             _r(q_sb[1][:, nsl]), start=False, stop=True)
                        pt = ptp.tile([128, 2 * NT], F32, tag="pt")
                        nc.scalar.activation(pt, ps, mybir.ActivationFunctionType.Exp)
                        first = mp == 0
                        last = mp == n_mp - 1
                        for j in range(2):
                            mb = 2 * mp + j
                            osl = slice(j * NT, j * NT + NT)
                            st = first and j == 0
                            sp = last and j == 1
                            nc.tensor.matmul(po[0], _r(vt_sb[mb][:, 0:128]), _r(pt[:, osl]),
                                             start=st, stop=sp, skip_group_check=True)
                            nc.tensor.matmul(po[1], _r(vt_sb[mb][:, 128:256]), _r(pt[:, osl]),
                                             start=st, stop=sp, skip_group_check=True)
                            nc.tensor.matmul(psum, _r(ones128), _r(pt[:, osl]),
                                             start=st, stop=sp, skip_group_check=True)

                    # epilogue for this n-tile
                    inv = ep.tile([128, NT], F32, tag="inv")
                    scr = ep.tile([128, NT], F32, tag="scr")
                    nc.vector.reciprocal_approx_accurate(inv, psum, scr)
                    ou = [ep.tile([128, NT], F32, tag=f"ou{cb}") for cb in range(2)]
                    nc.vector.tensor_mul(ou[0], po[0], inv)
                    nc.vector.tensor_mul(ou[1], po[1], inv)
                    for ob in range(2):
                        cs = slice(ob * 128, ob * 128 + 128)
                        pff = pf.tile([128, NT], F32, tag="pf")
                        nc.tensor.matmul(pff, _r(wo0[:, cs]), _r(ou[0]),
                                         start=True, stop=False)
                        nc.tensor.matmul(pff, _r(wo1[:, cs]), _r(ou[1]),
                                         start=False, stop=False)
                        nc.tensor.matmul(pff, _r(wob[0:1, cs]), _r(xqon[0:1, nsl]),
                                         start=False, stop=True)
                        fin = ep.tile([128, NT], F32, tag=f"fin{ob}")
                        xres = xq0 if ob == 0 else xq1
                        nc.vector.tensor_add(fin, pff, xres[:, nsl])
                        nc.sync.dma_start(out=y.ap()[cs, nsl], in_=fin)
    return nc


def _build():
    if "nc" not in _CACHE:
        nc = bacc.Bacc("TRN2", target_bir_lowering=False, debug=False, num_devices=8)
        _emit(nc)
        nc.compile()
        _CACHE["nc"] = nc
    return _CACHE["nc"]


def kernel(x, w_qkv, b_qkv, w_out, b_out):
    x = np.asarray(x, dtype=np.float32)
    w_qkv = np.asarray(w_qkv, dtype=np.float32)
    b_qkv = np.asarray(b_qkv, dtype=np.float32)
    w_out = np.asarray(w_out, dtype=np.float32)
    b_out = np.asarray(b_out, dtype=np.float32)

    nc = _build()

    scale = 1.0 / np.sqrt(C)
    waug = {}
    waug["wq"] = np.ascontiguousarray(
        np.vstack([w_qkv[0:C].T, b_qkv[0:C][None]]) * scale, dtype=np.float32)
    waug["wk"] = np.ascontiguousarray(
        np.vstack([w_qkv[C:2 * C].T, b_qkv[C:2 * C][None]]), dtype=np.float32)
    waug["wv"] = np.ascontiguousarray(
        np.vstack([w_qkv[2 * C:3 * C].T, b_qkv[2 * C:3 * C][None]]), dtype=np.float32)
    waug["wo"] = np.ascontiguousarray(
        np.vstack([w_out.T, b_out[None]]), dtype=np.float32)

    x4 = x.reshape(B, C, HW)
    in_maps = []
    for i in range(8):
        s, h = i // 2, i % 2
        xkv = np.empty((C + 1, HW), dtype=np.float32)
        xkv[0:C] = x4[s]
        xkv[C] = 1.0
        m = {"xkv": xkv,
             "xq": np.ascontiguousarray(xkv[:, h * NH:(h + 1) * NH])}
        m.update(waug)
        in_maps.append(m)

    res = run_bass_kernel_spmd(nc, in_maps, core_ids=list(range(8)))

    out = np.empty((B, C, HW), dtype=np.float32)
    for i in range(8):
        s, h = i // 2, i % 2
        out[s, :, h * NH:(h + 1) * NH] = res.results[i]["y"]
    return out.reshape(B, C, 64, 64)


# revision 5
# speedup vs baseline: 1.0631x; 1.0631x over previous
"""AttentionBlock kernel for 8 TRN2 NeuronCores.

Reference math (per sample s of 4, C=256 channels, HW=64*64=4096 positions):
  qkv = w_qkv @ x + b_qkv ; q,k,v = split(qkv)
  S = (q^T k) / sqrt(C) ; P = softmax(S, axis=-1)
  out = w_out @ (P @ v^T)^T + b_out + x

Sharding: core i -> (sample s=i//2, row half h=i%2, rows n0=h*2048 .. +2048).
K/V are computed for the full sample on both half-cores (duplicate compute is
cheap); Q and the attention rows only for the core's half.

On-chip layout: scores are computed transposed, S^T[m, n] (m = key position on
partitions, n = query row in free dim), so P^T = exp(S^T) is directly the
moving operand of the PV matmul (contraction over m = partitions) -- no
transposes anywhere. Softmax row sums come from an extra matmul with an
all-ones stationary operand (result is pre-broadcast across partitions);
normalization is folded into the PSUM->SBUF copy as a tensor*tensor multiply
with the reciprocal. Projection biases are folded into the matmuls by
augmenting x and the weights with a ones row (K=257); the 1/sqrt(C) scale is
folded into w_q on the host. All matmuls run in float32r (full PE rate).
"""

import sys

sys.path.insert(0, "/opt/trn_rl_repo")

import numpy as np

import concourse.bacc as bacc
import concourse.tile as tile
from concourse import mybir
from concourse.bass_utils import run_bass_kernel_spmd

B = 4
C = 256
HW = 4096  # 64*64
NH = 2048  # rows per core (half sample)
NT = 512   # n-tile (free dim per psum bank)
F32 = mybir.dt.float32
F32R = mybir.dt.float32r

_CACHE = {}


def _body(nc, pools):
    const, xp, qk, vtp, ptp, ep, pss, pacc, pf, dram = pools
    xkv, xq, wtens, y = dram

    # ---- load inputs ----
    xk0 = xp.tile([128, HW], F32R, tag="xk0", name="xk0")
    xk1 = xp.tile([128, HW], F32R, tag="xk1", name="xk1")
    xon = xp.tile([1, HW], F32R, tag="xon", name="xon")
    nc.sync.dma_start(out=xk0, in_=xkv.ap()[0:128, :])
    nc.sync.dma_start(out=xk1, in_=xkv.ap()[128:256, :])
    nc.sync.dma_start(out=xon, in_=xkv.ap()[256:257, :])

    xq0 = xp.tile([128, NH], F32R, tag="xq0", name="xq0")
    xq1 = xp.tile([128, NH], F32R, tag="xq1", name="xq1")
    xqon = xp.tile([1, NH], F32R, tag="xqon", name="xqon")
    nc.sync.dma_start(out=xq0, in_=xq.ap()[0:128, :])
    nc.sync.dma_start(out=xq1, in_=xq.ap()[128:256, :])
    nc.sync.dma_start(out=xqon, in_=xq.ap()[256:257, :])

    ws = {}
    for name in ("wq", "wk", "wv", "wo"):
        t = wtens[name]
        w0 = const.tile([128, C], F32R, tag=name + "0", name=name + "0")
        w1 = const.tile([128, C], F32R, tag=name + "1", name=name + "1")
        wb = const.tile([1, C], F32R, tag=name + "b", name=name + "b")
        nc.sync.dma_start(out=w0, in_=t.ap()[0:128, :])
        nc.sync.dma_start(out=w1, in_=t.ap()[128:256, :])
        nc.sync.dma_start(out=wb, in_=t.ap()[256:257, :])
        ws[name] = (w0, w1, wb)

    ones_f = const.tile([128, 128], F32, tag="ones_f", name="ones_f")
    nc.vector.memset(ones_f, 1.0)
    ones128 = const.tile([128, 128], F32R, tag="ones", name="ones")
    nc.vector.tensor_copy(ones128, ones_f)

    # ---- projections ----
    q_sb = [qk.tile([128, NH], F32R, tag=f"q{cb}", name=f"q{cb}") for cb in range(2)]
    k_sb = [qk.tile([128, HW], F32R, tag=f"k{cb}", name=f"k{cb}") for cb in range(2)]
    # vT[mb][128, C]: vT[m, c] = sum_ci x_aug[ci, m] * wv_aug[ci, c]
    vt_sb = [vtp.tile([128, C], F32R, tag=f"v{mb}", name=f"v{mb}")
             for mb in range(HW // 128)]

    w0, w1, wb = ws["wq"]
    for cb in range(2):
        for half in range(2):
            sl = slice(half * 1024, half * 1024 + 1024)
            ps = pss.tile([128, 1024], F32, tag="ps", name="ps_q")
            for j in range(2):
                s2 = slice(half * 1024 + j * 512, half * 1024 + j * 512 + 512)
                o2 = slice(j * 512, j * 512 + 512)
                cs = slice(cb * 128, cb * 128 + 128)
                nc.tensor.matmul(ps[:, o2], w0[:, cs], xq0[:, s2],
                                 start=True, stop=False)
                nc.tensor.matmul(ps[:, o2], w1[:, cs], xq1[:, s2],
                                 start=False, stop=False)
                nc.tensor.matmul(ps[:, o2], wb[0:1, cs], xqon[0:1, s2],
                                 start=False, stop=True)
            nc.vector.tensor_copy(q_sb[cb][:, sl], ps)

    w0, w1, wb = ws["wk"]
    for cb in range(2):
        for quad in range(4):
            sl = slice(quad * 1024, quad * 1024 + 1024)
            ps = pss.tile([128, 1024], F32, tag="ps", name="ps_k")
            for j in range(2):
                s2 = slice(quad * 1024 + j * 512, quad * 1024 + j * 512 + 512)
                o2 = slice(j * 512, j * 512 + 512)
                cs = slice(cb * 128, cb * 128 + 128)
                nc.tensor.matmul(ps[:, o2], w0[:, cs], xk0[:, s2],
                                 start=True, stop=False)
                nc.tensor.matmul(ps[:, o2], w1[:, cs], xk1[:, s2],
                                 start=False, stop=False)
                nc.tensor.matmul(ps[:, o2], wb[0:1, cs], xon[0:1, s2],
                                 start=False, stop=True)
            nc.scalar.copy(k_sb[cb][:, sl], ps)

    w0, w1, wb = ws["wv"]
    for mb in range(HW // 128):
        ms = slice(mb * 128, mb * 128 + 128)
        pv = pacc.tile([128, NT], F32, tag=f"po{mb % 2}", name="pv")
        nc.tensor.matmul(pv[:, 0:C], xk0[:, ms], w0, start=True, stop=False)
        nc.tensor.matmul(pv[:, 0:C], xk1[:, ms], w1, start=False, stop=False)
        nc.tensor.matmul(pv[:, 0:C], xon[0:1, ms], wb[0:1, :],
                         start=False, stop=True)
        if mb % 2 == 0:
            nc.vector.tensor_copy(vt_sb[mb], pv[:, 0:C])
        else:
            nc.scalar.copy(vt_sb[mb], pv[:, 0:C])

    # ---- attention main loop ----
    wo0, wo1, wob = ws["wo"]
    n_mp = HW // 256  # m-pairs of 128 rows each
    for nt in range(NH // NT):
        nsl = slice(nt * NT, nt * NT + NT)
        po = [pacc.tile([128, NT], F32, tag=f"po{cb}", name=f"po{cb}")
              for cb in range(2)]
        psum = pacc.tile([128, NT], F32, tag="psum", name="psum")
        for mp in range(n_mp):
            ps = pss.tile([128, 2 * NT], F32, tag="ps", name="ps_s")
            for j in range(2):  # two m-chunks per pair
                mb = 2 * mp + j
                msl = slice(mb * 128, mb * 128 + 128)
                osl = slice(j * NT, j * NT + NT)
                nc.tensor.matmul(ps[:, osl], k_sb[0][:, msl], q_sb[0][:, nsl],
                                 start=True, stop=False)
                nc.tensor.matmul(ps[:, osl], k_sb[1][:, msl], q_sb[1][:, nsl],
                                 start=False, stop=True)
            pt = ptp.tile([128, 2 * NT], F32R, tag="pt", name="pt")
            nc.scalar.activation(pt, ps, mybir.ActivationFunctionType.Exp)
            first = mp == 0
            last = mp == n_mp - 1
            for j in range(2):
                mb = 2 * mp + j
                osl = slice(j * NT, j * NT + NT)
                st = first and j == 0
                sp = last and j == 1
                nc.tensor.matmul(po[0], vt_sb[mb][:, 0:128], pt[:, osl],
                                 start=st, stop=sp, skip_group_check=True)
                nc.tensor.matmul(po[1], vt_sb[mb][:, 128:256], pt[:, osl],
                                 start=st, stop=sp, skip_group_check=True)
                nc.tensor.matmul(psum, ones128, pt[:, osl],
                                 start=st, stop=sp, skip_group_check=True)

        # epilogue for this n-tile
        inv = ep.tile([128, NT], F32, tag="inv", name="inv")
        scr = ep.tile([128, NT], F32, tag="scr", name="scr")
        nc.vector.reciprocal_approx_accurate(inv, psum, scr)
        ou = [ep.tile([128, NT], F32R, tag=f"ou{cb}", name=f"ou{cb}")
              for cb in range(2)]
        nc.vector.tensor_mul(ou[0], po[0], inv)
        nc.vector.tensor_mul(ou[1], po[1], inv)
        for ob in range(2):
            cs = slice(ob * 128, ob * 128 + 128)
            pff = pf.tile([128, NT], F32, tag="pf", name="pff")
            nc.tensor.matmul(pff, wo0[:, cs], ou[0], start=True, stop=False)
            nc.tensor.matmul(pff, wo1[:, cs], ou[1], start=False, stop=False)
            nc.tensor.matmul(pff, wob[0:1, cs], xqon[0:1, nsl],
                             start=False, stop=True)
            fin = ep.tile([128, NT], F32, tag=f"fin{ob}", name=f"fin{ob}")
            xres = xq0 if ob == 0 else xq1
            nc.vector.tensor_add(fin, pff, xres[:, nsl].bitcast(F32))
            nc.sync.dma_start(out=y.ap()[cs, nsl], in_=fin)


def _emit(nc, reps=0):
    xkv = nc.dram_tensor("xkv", (C + 1, HW), F32R, kind="ExternalInput")
    xq = nc.dram_tensor("xq", (C + 1, NH), F32R, kind="ExternalInput")
    wtens = {n: nc.dram_tensor(n, (C + 1, C), F32R, kind="ExternalInput")
             for n in ("wq", "wk", "wv", "wo")}
    y = nc.dram_tensor("y", (C, NH), F32, kind="ExternalOutput")
    dram = (xkv, xq, wtens, y)

    with tile.TileContext(nc) as tc:
        with (
            tc.tile_pool(name="const", bufs=1) as const,
            tc.tile_pool(name="xp", bufs=1) as xp,
            tc.tile_pool(name="qk", bufs=1) as qk,
            tc.tile_pool(name="vt", bufs=1) as vtp,
            tc.tile_pool(name="pt", bufs=4) as ptp,
            tc.tile_pool(name="ep", bufs=2) as ep,
            tc.tile_pool(name="pss", bufs=2, space="PSUM") as pss,
            tc.tile_pool(name="pacc", bufs=1, space="PSUM") as pacc,
            tc.tile_pool(name="pf", bufs=1, space="PSUM") as pf,
        ):
            pools = (const, xp, qk, vtp, ptp, ep, pss, pacc, pf, dram)
            if reps:
                with tc.For_i(0, reps, 1, hint_engines=(
                        mybir.EngineType.PE, mybir.EngineType.Activation,
                        mybir.EngineType.DVE)):
                    _body(nc, pools)
            else:
                _body(nc, pools)
    return nc


def _build(reps=0):
    key = ("nc", reps)
    if key not in _CACHE:
        nc = bacc.Bacc("TRN2", target_bir_lowering=False, debug=False,
                       num_devices=8)
        _emit(nc, reps=reps)
        nc.compile()
        _CACHE[key] = nc
    return _CACHE[key]


def make_in_maps(x, w_qkv, b_qkv, w_out, b_out):
    scale = 1.0 / np.sqrt(C)
    waug = {
        "wq": np.ascontiguousarray(
            np.vstack([w_qkv[0:C].T, b_qkv[0:C][None]]) * scale,
            dtype=np.float32),
        "wk": np.ascontiguousarray(
            np.vstack([w_qkv[C:2 * C].T, b_qkv[C:2 * C][None]]),
            dtype=np.float32),
        "wv": np.ascontiguousarray(
            np.vstack([w_qkv[2 * C:3 * C].T, b_qkv[2 * C:3 * C][None]]),
            dtype=np.float32),
        "wo": np.ascontiguousarray(
            np.vstack([w_out.T, b_out[None]]), dtype=np.float32),
    }
    x4 = x.reshape(B, C, HW)
    in_maps = []
    for i in range(8):
        s, h = i // 2, i % 2
        xkv = np.empty((C + 1, HW), dtype=np.float32)
        xkv[0:C] = x4[s]
        xkv[C] = 1.0
        m = {"xkv": xkv,
             "xq": np.ascontiguousarray(xkv[:, h * NH:(h + 1) * NH])}
        m.update(waug)
        in_maps.append(m)
    return in_maps


def kernel(x, w_qkv, b_qkv, w_out, b_out):
    x = np.asarray(x, dtype=np.float32)
    w_qkv = np.asarray(w_qkv, dtype=np.float32)
    b_qkv = np.asarray(b_qkv, dtype=np.float32)
    w_out = np.asarray(w_out, dtype=np.float32)
    b_out = np.asarray(b_out, dtype=np.float32)

    nc = _build()
    in_maps = make_in_maps(x, w_qkv, b_qkv, w_out, b_out)
    res = run_bass_kernel_spmd(nc, in_maps, core_ids=list(range(8)))

    out = np.empty((B, C, HW), dtype=np.float32)
    for i in range(8):
        s, h = i // 2, i % 2
        out[s, :, h * NH:(h + 1) * NH] = res.results[i]["y"]
    return out.reshape(B, C, 64, 64)
